# revision 31
# baseline (speedup 1.0000x reference)
"""Trainium2 Bass kernel for a LeakyReLU RNN.

Model (B=128, S=512, I=256, H=1024, O=256):
    xproj = lrelu(x @ Wi.T + bi)                          # [B,S,H]
    h_t   = lrelu(concat(xproj_t, h_{t-1}) @ Wh.T + bh)   # recurrence over S
    out   = h_S @ Wo.T + bo                               # [B,O]

Strategy: data-parallel over batch (16 rows/core on 8 cores). Split
Wh = [Wh1 | Wh2]; U = xproj @ Wh1.T + bh is precomputed blockwise into
SBUF, the sequential part is h_t = lrelu(U_t + h_{t-1} @ Wh2.T).

The recurrence matmuls run in bf16 in 128x32 column-tiled PE mode:
three concurrent tiles (TRN2 forbids matmul dst partition 96), each
with its own moving-operand stream, so the per-step Wh2 stream is
~3072 cycles instead of 8192. Tile j covers output features
[384j, min(384j+384, 1024)) at PSUM partitions [32j, 32j+16). Phase-1
GEMM work for block b+2 is interleaved between the recurrence steps of
window b as PE filler, hiding phase 1 and keeping the PE p-state high.
bf16 is safe here: products accumulate in fp32 PSUM, U is injected
from an fp32-accumulated GEMM, and the recurrence is contractive
(||Wh2|| ~ 0.8), so the per-step ~4e-3 quantization error settles
around 1e-2, within the 2e-2 gate.
"""

from contextlib import ExitStack

import ml_dtypes
import numpy as np

import concourse.bacc as bacc
import concourse.tile as tile
from concourse import mybir
from concourse.bass_utils import run_bass_kernel_spmd

B, S, I, H, O = 128, 512, 256, 1024, 256
NCORES = 8
BL = B // NCORES          # batch rows per core = 16
TOK = BL * S              # tokens per core = 8192
ALPHA = 0.01
NSPLIT = (0, 384, 768, 1024)   # 3-way output-feature split for col tiles
# hT slot order: chunks whose transpose source lies in hn cols [0,256)
# first (they unblock after the first activation half), then the rest
KORDER = (0, 1, 3, 4, 6, 7, 2, 5)
import os
# ACT_SPLIT hangs the hardware: a partial-column activation read of a PSUM
# bank holding an active accumulation group never completes. Keep it off.
ACT_SPLIT = os.environ.get("ACT_SPLIT", "0") == "1"    # lrelu in two halves
COPY_SPLIT = os.environ.get("COPY_SPLIT", "0") == "1"  # psT -> ht in 2 copies
USE_KORDER = os.environ.get("USE_KORDER", "0") == "1"  # completion-order slots

F32 = mybir.dt.float32
F32R = mybir.dt.float32r
BF16 = mybir.dt.bfloat16
LRELU = mybir.ActivationFunctionType.Lrelu

_CACHED = None


def _build(S=S):
    TOK = BL * S
    NBLK = TOK // 512                 # 512-token (32-step) phase-1 blocks
    nc = bacc.Bacc("TRN2", target_bir_lowering=False, debug=False,
                   num_devices=NCORES)

    xt_d = nc.dram_tensor("xt", [I, TOK], F32, kind="ExternalInput")
    wit_d = nc.dram_tensor("wit", [I, H], F32, kind="ExternalInput")
    wh1t_d = nc.dram_tensor("wh1t", [H, H], F32, kind="ExternalInput")
    wh2b_d = nc.dram_tensor("wh2b", [H, H], BF16, kind="ExternalInput")
    wob_d = nc.dram_tensor("wob", [H, O], BF16, kind="ExternalInput")
    bi_d = nc.dram_tensor("bi", [128, H // 128], F32, kind="ExternalInput")
    bhrep_d = nc.dram_tensor("bhrep", [128, H], F32, kind="ExternalInput")
    borep_d = nc.dram_tensor("borep", [128, O], F32, kind="ExternalInput")
    eyeb_d = nc.dram_tensor("eyeb", [128, 128], BF16, kind="ExternalInput")
    y_d = nc.dram_tensor("y", [BL, O], F32, kind="ExternalOutput")

    with tile.TileContext(nc) as tc, ExitStack() as ctx:
        wpool = ctx.enter_context(tc.tile_pool(name="weights", bufs=1))
        xtpool = ctx.enter_context(tc.tile_pool(name="xt", bufs=2))
        apool = ctx.enter_context(tc.tile_pool(name="atiles", bufs=2))
        upool = ctx.enter_context(tc.tile_pool(name="usb", bufs=2))
        hnpool = ctx.enter_context(tc.tile_pool(name="hn", bufs=2))
        htpool = ctx.enter_context(tc.tile_pool(name="ht", bufs=2))
        opool = ctx.enter_context(tc.tile_pool(name="osb", bufs=1))
        psA = ctx.enter_context(tc.tile_pool(name="psA", bufs=2, space="PSUM"))
        psU = ctx.enter_context(tc.tile_pool(name="psU", bufs=1, space="PSUM"))
        psR = ctx.enter_context(tc.tile_pool(name="psR", bufs=2, space="PSUM"))
        psT = ctx.enter_context(tc.tile_pool(name="psT", bufs=2, space="PSUM"))

        # ---- resident weights ----
        def wload(src, shape, tag, dt=F32R, eng=None):
            t = wpool.tile(shape, dt, tag=tag, name=tag)
            (eng or nc.gpsimd).dma_start(t[:], src)
            return t

        wit = [wload(wit_d.ap()[128 * k:128 * (k + 1), :], [128, H], f"wit{k}")
               for k in range(2)]
        wh1t = [wload(wh1t_d.ap()[128 * k:128 * (k + 1), :], [128, H], f"wh1t{k}")
                for k in range(8)]
        wh2b = [wload(wh2b_d.ap()[128 * k:128 * (k + 1), :], [128, H],
                      f"wh2b{k}", dt=BF16, eng=nc.sync)
                for k in range(8)]
        wob = [wload(wob_d.ap()[128 * k:128 * (k + 1), :], [128, O],
                     f"wob{k}", dt=BF16, eng=nc.sync)
               for k in range(8)]
        eyeb = wload(eyeb_d.ap(), [128, 128], "eyeb", dt=BF16, eng=nc.sync)
        bhrep = wload(bhrep_d.ap(), [128, H], "bhrep", dt=F32, eng=nc.sync)
        borep = wload(borep_d.ap(), [128, O], "borep", dt=F32, eng=nc.sync)
        bi = wpool.tile([128, H // 128], F32, tag="bi", name="bi")
        nc.sync.dma_start(bi[:], bi_d.ap())

        # ---- phase 1 (emitted as filler chunks between recurrence steps)
        # Per 512-token block: A_T = lrelu(WiT.T @ Xt + bi)  (feature-major)
        # then U_g = A_g @ Wh1.T + bh for 4 groups of 128 tokens
        # (token-major [128 tok, 1024] bf16), kept in SBUF for the recurrence.
        usb = {}   # (blk % 2, g) -> SBUF tile [128, H]

        def phase1_chunks(blk, t_base):
            """Return a FIFO of (gate, closure); each closure emits
            ~200-450ns of engine work. A closure may only be emitted at
            absolute step >= gate: the usb evacuations overwrite (pool-wise)
            the buffer the current window's injects still read, so they are
            gated past the last consumer; the psU accumulations for the next
            group must stay behind the previous group's evacuation."""
            c0 = 512 * blk
            chunks = []
            xt = []
            a = []

            def dma_chunk():
                for k in range(2):
                    t = xtpool.tile([128, 512], F32R, tag=f"xt{k}",
                                    name=f"xt{k}_{blk}")
                    nc.gpsimd.dma_start(
                        t[:], xt_d.ap()[128 * k:128 * (k + 1), c0:c0 + 512])
                    xt.append(t)
            chunks.append((0, dma_chunk))

            def a_chunk(m):
                pa = psA.tile([128, 512], F32, tag="psA", name=f"psA_{blk}_{m}")
                nc.tensor.matmul(pa[:], wit[0][:, 128 * m:128 * (m + 1)],
                                 xt[0][:], start=True, stop=False)
                nc.tensor.matmul(pa[:], wit[1][:, 128 * m:128 * (m + 1)],
                                 xt[1][:], start=False, stop=True)
                am = apool.tile([128, 512], F32R, tag=f"a{m}", name=f"a{m}_{blk}")
                nc.scalar.activation(am[:], pa[:], LRELU,
                                     bias=bi[:, m:m + 1], scale=1.0, alpha=ALPHA)
                a.append(am)
            for m in range(8):
                chunks.append((0, lambda m=m: a_chunk(m)))

            # U for group g accumulates into psU [128 tok, 1024]
            # (each matmul dst must stay within one 2KB PSUM bank -> n-halves)
            pu = {}

            def u_chunk(g, k):
                if k == 0:
                    pu[g] = psU.tile([128, H], F32, tag="psU",
                                     name=f"psU_{blk}_{g}")
                for n in range(2):
                    nc.tensor.matmul(pu[g][:, 512 * n:512 * (n + 1)],
                                     a[k][:, 128 * g:128 * (g + 1)],
                                     wh1t[k][:, 512 * n:512 * (n + 1)],
                                     start=(k == 0), stop=(k == 7))

            def u_evac(g, q):
                # evacuate in 256-col slices so the DVE never blocks the
                # recurrence's critical hT copy for long; bf16 cast here
                if q == 0:
                    usb[(blk, g)] = upool.tile([128, H], BF16,
                                               tag=f"usb{g}",
                                               name=f"usb{g}_{blk}")
                sl = slice(256 * q, 256 * (q + 1))
                nc.vector.tensor_add(usb[(blk, g)][:, sl], pu[g][:, sl],
                                     bhrep[:, sl])

            for g in range(4):
                # psU has a single buffer: group g's matmuls may only be
                # emitted once group g-1's evacuation is fully emitted, which
                # the FIFO order plus the evac gates below guarantee.
                for k in range(8):
                    chunks.append((0, lambda g=g, k=k: u_chunk(g, k)))
                for q in range(4):
                    # the usb{g} buffer this evac overwrites is read by the
                    # current window's g-group injects (local steps 8g..8g+7)
                    chunks.append((t_base + 8 * g + 7,
                                   lambda g=g, q=q: u_evac(g, q)))
            return chunks

        # ---- recurrence state ----
        ht = htpool.tile([128, 128], BF16, tag="hT", name="hT_init")
        nc.gpsimd.memset(ht[:].bitcast(F32), 0.0)

        def step(t):
            """One recurrence step: 3 col-tiles of bf16 matmuls + transposes.

            Col tile j covers features [NSPLIT[j], NSPLIT[j+1]) and writes
            PSUM partitions [32j, 32j+16)."""
            nonlocal ht
            blk, g, s = t // 32, (t % 32) // 8, t % 8
            ut = usb[(blk, g)]
            ps = psR.tile([128, 384], F32, tag="psR", name=f"psR_{t}")
            sel = eyeb[:, 16 * s:16 * (s + 1)]
            for j in range(3):
                w = NSPLIT[j + 1] - NSPLIT[j]
                nc.tensor.matmul(ps[32 * j:32 * j + 16, 0:w], sel,
                                 ut[:, NSPLIT[j]:NSPLIT[j + 1]],
                                 start=True, stop=False,
                                 tile_position=(0, 32 * j))
            for o, k in enumerate(KORDER if USE_KORDER else range(8)):
                hk = ht[:, 16 * o:16 * (o + 1)]   # slot o holds chunk k
                for j in range(3):
                    w = NSPLIT[j + 1] - NSPLIT[j]
                    nc.tensor.matmul(ps[32 * j:32 * j + 16, 0:w], hk,
                                     wh2b[k][:, NSPLIT[j]:NSPLIT[j + 1]],
                                     start=False, stop=(o == 7),
                                     tile_position=(0, 32 * j))
            # lrelu split in two column halves so the first six transposes
            # (and the next step's first matmuls) start ~300ns earlier
            hn = hnpool.tile([128, 384], BF16, tag="hn", name=f"hn_{t}")
            if ACT_SPLIT:
                nc.scalar.activation(hn[:, 0:256], ps[:, 0:256], LRELU,
                                     bias=0.0, scale=1.0, alpha=ALPHA)
                nc.scalar.activation(hn[:, 256:384], ps[:, 256:384], LRELU,
                                     bias=0.0, scale=1.0, alpha=ALPHA)
            else:
                nc.scalar.activation(hn[:], ps[:], LRELU,
                                     bias=0.0, scale=1.0, alpha=ALPHA)
            # transpose back to feature-major stationary for the next step,
            # in completion order: chunks fed by the first act half first
            # (psT/ht slot o holds feature chunk KORDER[o])
            pt = psT.tile([128, 128], BF16, tag="psT", name=f"psT_{t}")
            for o, k in enumerate(KORDER if USE_KORDER else range(8)):
                j = (128 * k) // 384          # source col tile
                c = 128 * k - 384 * j         # col offset within tile j
                nc.tensor.transpose(
                    pt[:, 16 * o:16 * (o + 1)],
                    hn[32 * j:32 * j + 16, c:c + 128],
                    eyeb[32 * j:32 * j + 16, 32 * j:32 * j + 16],
                    tile_position=(32 * j, 0))
            ht_new = htpool.tile([128, 128], BF16, tag="hT", name=f"hT_{t}")
            if COPY_SPLIT:
                nc.vector.tensor_copy(ht_new[:, 0:96], pt[:, 0:96])
                nc.vector.tensor_copy(ht_new[:, 96:128], pt[:, 96:128])
            else:
                nc.vector.tensor_copy(ht_new[:], pt[:])
            ht = ht_new

        # ---- schedule: prologue blocks 0,1 then steps with filler ----
        for _, c in phase1_chunks(0, 0):
            c()
        if NBLK > 1:
            for _, c in phase1_chunks(1, 0):
                c()
        def dummy_chunk(t, i):
            # keep-warm matmul into a scratch PSUM tile: holds the PE at its
            # high p-state through steps that have no phase-1 filler left
            pa = psA.tile([128, 512], F32, tag="psA", name=f"dummy_{t}_{i}")
            nc.tensor.matmul(pa[:], wit[0][:, 0:128], wh1t[0][:, 0:512],
                             start=True, stop=True)

        queue = []
        for t in range(S):
            if t % 32 == 0:
                nblk = t // 32 + 2
                if nblk < NBLK:
                    queue.extend(phase1_chunks(nblk, t))
            step(t)
            # emit filler chunks to keep the PE busy during the act/copy gap
            budget = 3
            emitted = 0
            while queue and budget > 0 and queue[0][0] <= t:
                queue.pop(0)[1]()
                budget -= 1
                emitted += 1
            if emitted == 0 and t + 1 < S:
                for i in range(2):
                    dummy_chunk(t, i)

        # ---- phase 3: out = h_S @ Wo.T + bo ----
        po = psR.tile([128, 384], F32, tag="psR", name="psO")
        for o, k in enumerate(KORDER if USE_KORDER else range(8)):
            nc.tensor.matmul(po[0:16, 0:O], ht[:, 16 * o:16 * (o + 1)],
                             wob[k][:], start=(o == 0), stop=(o == 7))
        osb = opool.tile([16, O], F32, tag="osb", name="osb")
        nc.vector.tensor_add(osb[:], po[0:16, 0:O], borep[0:16, :])
        nc.sync.dma_start(y_d.ap(), osb[:])

    nc.compile()
    return nc


def _prep_inputs(x, Wi, bi, Wh, bh, Wo, bo):
    bf = ml_dtypes.bfloat16
    shared = {
        "wit": np.ascontiguousarray(Wi.T),
        "wh1t": np.ascontiguousarray(Wh[:, :H].T),
        "wh2b": np.ascontiguousarray(Wh[:, H:].T).astype(bf),
        "wob": np.ascontiguousarray(Wo.T).astype(bf),
        "bi": np.ascontiguousarray(bi.reshape(H // 128, 128).T),
        "bhrep": np.ascontiguousarray(np.broadcast_to(bh.reshape(1, H),
                                                      (128, H))),
        "borep": np.ascontiguousarray(np.broadcast_to(bo.reshape(1, O),
                                                      (128, O))),
        "eyeb": np.eye(128, dtype=np.float32).astype(bf),
    }
    in_maps = []
    for c in range(NCORES):
        xc = x[BL * c:BL * (c + 1)]            # [16, S, I]
        xt = np.ascontiguousarray(
            xc.transpose(2, 1, 0).reshape(I, xc.shape[1] * BL))
        m = dict(shared)
        m["xt"] = xt
        in_maps.append(m)
    return in_maps


def kernel(x, Wi, bi, Wh, bh, Wo, bo, _trace=False):
    global _CACHED
    x = np.asarray(x, dtype=np.float32)
    if _CACHED is None:
        _CACHED = _build()
    nc = _CACHED
    in_maps = _prep_inputs(np.asarray(x, np.float32), np.asarray(Wi, np.float32),
                           np.asarray(bi, np.float32), np.asarray(Wh, np.float32),
                           np.asarray(bh, np.float32), np.asarray(Wo, np.float32),
                           np.asarray(bo, np.float32))
    res = run_bass_kernel_spmd(nc, in_maps, list(range(NCORES)), trace=_trace)
    out = np.concatenate([res.results[c]["y"] for c in range(NCORES)], axis=0)
    if _trace:
        return out, res
    return out


# revision 32
# speedup vs baseline: 1.0239x; 1.0239x over previous
"""Trainium2 Bass kernel for a LeakyReLU RNN.

Model (B=128, S=512, I=256, H=1024, O=256):
    xproj = lrelu(x @ Wi.T + bi)                          # [B,S,H]
    h_t   = lrelu(concat(xproj_t, h_{t-1}) @ Wh.T + bh)   # recurrence over S
    out   = h_S @ Wo.T + bo                               # [B,O]

Strategy: data-parallel over batch (16 rows/core on 8 cores). Split
Wh = [Wh1 | Wh2]; U = xproj @ Wh1.T + bh is precomputed blockwise into
SBUF, the sequential part is h_t = lrelu(U_t + h_{t-1} @ Wh2.T).

The recurrence matmuls run in bf16 in 128x32 column-tiled PE mode:
three concurrent tiles (TRN2 forbids matmul dst partition 96), each
with its own moving-operand stream, so the per-step Wh2 stream is
~3072 cycles instead of 8192. Tile j covers output features
[384j, min(384j+384, 1024)) at PSUM partitions [32j, 32j+16). Phase-1
GEMM work for block b+2 is interleaved between the recurrence steps of
window b as PE filler, hiding phase 1 and keeping the PE p-state high.
bf16 is safe here: products accumulate in fp32 PSUM, U is injected
from an fp32-accumulated GEMM, and the recurrence is contractive
(||Wh2|| ~ 0.8), so the per-step ~4e-3 quantization error settles
around 1e-2, within the 2e-2 gate.
"""

from contextlib import ExitStack

import ml_dtypes
import numpy as np

import concourse.bacc as bacc
import concourse.tile as tile
from concourse import mybir
from concourse.bass_utils import run_bass_kernel_spmd

B, S, I, H, O = 128, 512, 256, 1024, 256
NCORES = 8
BL = B // NCORES          # batch rows per core = 16
TOK = BL * S              # tokens per core = 8192
ALPHA = 0.01
NSPLIT = (0, 384, 768, 1024)   # 3-way output-feature split for col tiles
# hT slot order: chunks whose transpose source lies in hn cols [0,256)
# first (they unblock after the first activation half), then the rest
KORDER = (0, 1, 3, 4, 6, 7, 2, 5)
import os
# ACT_SPLIT hangs the hardware: a partial-column activation read of a PSUM
# bank holding an active accumulation group never completes. Keep it off.
ACT_SPLIT = os.environ.get("ACT_SPLIT", "0") == "1"    # lrelu in two halves
COPY_SPLIT = os.environ.get("COPY_SPLIT", "0") == "1"  # psT -> ht in 2 copies
USE_KORDER = os.environ.get("USE_KORDER", "0") == "1"  # completion-order slots

F32 = mybir.dt.float32
F32R = mybir.dt.float32r
BF16 = mybir.dt.bfloat16
LRELU = mybir.ActivationFunctionType.Lrelu

_CACHED = None


def _build(S=S):
    TOK = BL * S
    NBLK = TOK // 512                 # 512-token (32-step) phase-1 blocks
    nc = bacc.Bacc("TRN2", target_bir_lowering=False, debug=False,
                   num_devices=NCORES)

    xt_d = nc.dram_tensor("xt", [I, TOK], F32, kind="ExternalInput")
    wit_d = nc.dram_tensor("wit", [I, H], F32, kind="ExternalInput")
    wh1t_d = nc.dram_tensor("wh1t", [H, H], F32, kind="ExternalInput")
    wh2b_d = nc.dram_tensor("wh2b", [H, H], BF16, kind="ExternalInput")
    wob_d = nc.dram_tensor("wob", [H, O], BF16, kind="ExternalInput")
    bi_d = nc.dram_tensor("bi", [128, H // 128], F32, kind="ExternalInput")
    bhrep_d = nc.dram_tensor("bhrep", [128, H], F32, kind="ExternalInput")
    borep_d = nc.dram_tensor("borep", [128, O], F32, kind="ExternalInput")
    eyeb_d = nc.dram_tensor("eyeb", [128, 128], BF16, kind="ExternalInput")
    y_d = nc.dram_tensor("y", [BL, O], F32, kind="ExternalOutput")

    with tile.TileContext(nc) as tc, ExitStack() as ctx:
        wpool = ctx.enter_context(tc.tile_pool(name="weights", bufs=1))
        xtpool = ctx.enter_context(tc.tile_pool(name="xt", bufs=2))
        apool = ctx.enter_context(tc.tile_pool(name="atiles", bufs=2))
        upool = ctx.enter_context(tc.tile_pool(name="usb", bufs=2))
        hnpool = ctx.enter_context(tc.tile_pool(name="hn", bufs=2))
        htpool = ctx.enter_context(tc.tile_pool(name="ht", bufs=2))
        opool = ctx.enter_context(tc.tile_pool(name="osb", bufs=1))
        psA = ctx.enter_context(tc.tile_pool(name="psA", bufs=2, space="PSUM"))
        psU = ctx.enter_context(tc.tile_pool(name="psU", bufs=1, space="PSUM"))
        psR = ctx.enter_context(tc.tile_pool(name="psR", bufs=2, space="PSUM"))
        psT = ctx.enter_context(tc.tile_pool(name="psT", bufs=2, space="PSUM"))

        # ---- resident weights ----
        def wload(src, shape, tag, dt=F32R, eng=None):
            t = wpool.tile(shape, dt, tag=tag, name=tag)
            (eng or nc.gpsimd).dma_start(t[:], src)
            return t

        wit = [wload(wit_d.ap()[128 * k:128 * (k + 1), :], [128, H], f"wit{k}")
               for k in range(2)]
        wh1t = [wload(wh1t_d.ap()[128 * k:128 * (k + 1), :], [128, H], f"wh1t{k}")
                for k in range(8)]
        wh2b = [wload(wh2b_d.ap()[128 * k:128 * (k + 1), :], [128, H],
                      f"wh2b{k}", dt=BF16, eng=nc.sync)
                for k in range(8)]
        wob = [wload(wob_d.ap()[128 * k:128 * (k + 1), :], [128, O],
                     f"wob{k}", dt=BF16, eng=nc.sync)
               for k in range(8)]
        eyeb = wload(eyeb_d.ap(), [128, 128], "eyeb", dt=BF16, eng=nc.sync)
        bhrep = wload(bhrep_d.ap(), [128, H], "bhrep", dt=F32, eng=nc.sync)
        borep = wload(borep_d.ap(), [128, O], "borep", dt=F32, eng=nc.sync)
        bi = wpool.tile([128, H // 128], F32, tag="bi", name="bi")
        nc.sync.dma_start(bi[:], bi_d.ap())

        # ---- phase 1 (emitted as filler chunks between recurrence steps)
        # Per 512-token block: A_T = lrelu(WiT.T @ Xt + bi)  (feature-major)
        # then U_g = A_g @ Wh1.T + bh for 4 groups of 128 tokens
        # (token-major [128 tok, 1024] bf16), kept in SBUF for the recurrence.
        usb = {}   # (blk % 2, g) -> SBUF tile [128, H]

        def phase1_chunks(blk, t_base):
            """Return a FIFO of (gate, closure); each closure emits
            ~200-450ns of engine work. A closure may only be emitted at
            absolute step >= gate: the usb evacuations overwrite (pool-wise)
            the buffer the current window's injects still read, so they are
            gated past the last consumer; the psU accumulations for the next
            group must stay behind the previous group's evacuation."""
            c0 = 512 * blk
            chunks = []
            xt = []
            a = []

            def dma_chunk():
                for k in range(2):
                    t = xtpool.tile([128, 512], F32R, tag=f"xt{k}",
                                    name=f"xt{k}_{blk}")
                    nc.gpsimd.dma_start(
                        t[:], xt_d.ap()[128 * k:128 * (k + 1), c0:c0 + 512])
                    xt.append(t)
            chunks.append((0, dma_chunk))

            def a_chunk(m):
                pa = psA.tile([128, 512], F32, tag="psA", name=f"psA_{blk}_{m}")
                nc.tensor.matmul(pa[:], wit[0][:, 128 * m:128 * (m + 1)],
                                 xt[0][:], start=True, stop=False)
                nc.tensor.matmul(pa[:], wit[1][:, 128 * m:128 * (m + 1)],
                                 xt[1][:], start=False, stop=True)
                am = apool.tile([128, 512], F32R, tag=f"a{m}", name=f"a{m}_{blk}")
                nc.scalar.activation(am[:], pa[:], LRELU,
                                     bias=bi[:, m:m + 1], scale=1.0, alpha=ALPHA)
                a.append(am)
            for m in range(8):
                chunks.append((0, lambda m=m: a_chunk(m)))

            # U for group g accumulates into psU [128 tok, 1024]
            # (each matmul dst must stay within one 2KB PSUM bank -> n-halves)
            pu = {}

            def u_chunk(g, k):
                if k == 0:
                    pu[g] = psU.tile([128, H], F32, tag="psU",
                                     name=f"psU_{blk}_{g}")
                for n in range(2):
                    nc.tensor.matmul(pu[g][:, 512 * n:512 * (n + 1)],
                                     a[k][:, 128 * g:128 * (g + 1)],
                                     wh1t[k][:, 512 * n:512 * (n + 1)],
                                     start=(k == 0), stop=(k == 7))

            def u_evac(g, q):
                # evacuate in 256-col slices so the DVE never blocks the
                # recurrence's critical hT copy for long; bf16 cast here
                if q == 0:
                    usb[(blk, g)] = upool.tile([128, H], BF16,
                                               tag=f"usb{g}",
                                               name=f"usb{g}_{blk}")
                sl = slice(256 * q, 256 * (q + 1))
                nc.vector.tensor_add(usb[(blk, g)][:, sl], pu[g][:, sl],
                                     bhrep[:, sl])

            for g in range(4):
                # psU has a single buffer: group g's matmuls may only be
                # emitted once group g-1's evacuation is fully emitted, which
                # the FIFO order plus the evac gates below guarantee.
                for k in range(8):
                    chunks.append((0, lambda g=g, k=k: u_chunk(g, k)))
                for q in range(4):
                    # the usb{g} buffer this evac overwrites is read by the
                    # current window's g-group injects (local steps 8g..8g+7)
                    chunks.append((t_base + 8 * g + 7,
                                   lambda g=g, q=q: u_evac(g, q)))
            return chunks

        # ---- recurrence state ----
        ht = htpool.tile([128, 128], BF16, tag="hT", name="hT_init")
        nc.gpsimd.memset(ht[:].bitcast(F32), 0.0)

        def step(t):
            """One recurrence step: 3 col-tiles of bf16 matmuls + transposes.

            Col tile j covers features [NSPLIT[j], NSPLIT[j+1]) and writes
            PSUM partitions [32j, 32j+16)."""
            nonlocal ht
            blk, g, s = t // 32, (t % 32) // 8, t % 8
            ut = usb[(blk, g)]
            ps = psR.tile([128, 384], F32, tag="psR", name=f"psR_{t}")
            sel = eyeb[:, 16 * s:16 * (s + 1)]
            for j in range(3):
                w = NSPLIT[j + 1] - NSPLIT[j]
                nc.tensor.matmul(ps[32 * j:32 * j + 16, 0:w], sel,
                                 ut[:, NSPLIT[j]:NSPLIT[j + 1]],
                                 start=True, stop=False,
                                 tile_position=(0, 32 * j))
            for o, k in enumerate(KORDER if USE_KORDER else range(8)):
                hk = ht[:, 16 * o:16 * (o + 1)]   # slot o holds chunk k
                for j in range(3):
                    w = NSPLIT[j + 1] - NSPLIT[j]
                    nc.tensor.matmul(ps[32 * j:32 * j + 16, 0:w], hk,
                                     wh2b[k][:, NSPLIT[j]:NSPLIT[j + 1]],
                                     start=False, stop=(o == 7),
                                     tile_position=(0, 32 * j))
            # lrelu split in two column halves so the first six transposes
            # (and the next step's first matmuls) start ~300ns earlier
            hn = hnpool.tile([128, 384], BF16, tag="hn", name=f"hn_{t}")
            if ACT_SPLIT:
                nc.scalar.activation(hn[:, 0:256], ps[:, 0:256], LRELU,
                                     bias=0.0, scale=1.0, alpha=ALPHA)
                nc.scalar.activation(hn[:, 256:384], ps[:, 256:384], LRELU,
                                     bias=0.0, scale=1.0, alpha=ALPHA)
            else:
                nc.scalar.activation(hn[:], ps[:], LRELU,
                                     bias=0.0, scale=1.0, alpha=ALPHA)
            # transpose back to feature-major stationary for the next step,
            # in completion order: chunks fed by the first act half first
            # (psT/ht slot o holds feature chunk KORDER[o])
            pt = psT.tile([128, 128], BF16, tag="psT", name=f"psT_{t}")
            for o, k in enumerate(KORDER if USE_KORDER else range(8)):
                j = (128 * k) // 384          # source col tile
                c = 128 * k - 384 * j         # col offset within tile j
                nc.tensor.transpose(
                    pt[:, 16 * o:16 * (o + 1)],
                    hn[32 * j:32 * j + 16, c:c + 128],
                    eyeb[32 * j:32 * j + 16, 32 * j:32 * j + 16],
                    tile_position=(32 * j, 0))
            ht_new = htpool.tile([128, 128], BF16, tag="hT", name=f"hT_{t}")
            if COPY_SPLIT:
                nc.vector.tensor_copy(ht_new[:, 0:96], pt[:, 0:96])
                nc.vector.tensor_copy(ht_new[:, 96:128], pt[:, 96:128])
            else:
                nc.vector.tensor_copy(ht_new[:], pt[:])
            ht = ht_new

        # ---- schedule: prologue blocks 0,1 then steps with filler ----
        for _, c in phase1_chunks(0, 0):
            c()
        if NBLK > 1:
            for _, c in phase1_chunks(1, 0):
                c()
        queue = []
        for t in range(S):
            if t % 32 == 0:
                nblk = t // 32 + 2
                if nblk < NBLK:
                    queue.extend(phase1_chunks(nblk, t))
            step(t)
            # emit filler chunks to keep the PE busy during the act/copy
            # gap; cap at 2/step (avg demand is 1.78) so the insertions stay
            # smooth instead of front-loading each 32-step window
            budget = 2
            while queue and budget > 0 and queue[0][0] <= t:
                queue.pop(0)[1]()
                budget -= 1

        # ---- phase 3: out = h_S @ Wo.T + bo ----
        po = psR.tile([128, 384], F32, tag="psR", name="psO")
        for o, k in enumerate(KORDER if USE_KORDER else range(8)):
            nc.tensor.matmul(po[0:16, 0:O], ht[:, 16 * o:16 * (o + 1)],
                             wob[k][:], start=(o == 0), stop=(o == 7))
        osb = opool.tile([16, O], F32, tag="osb", name="osb")
        nc.vector.tensor_add(osb[:], po[0:16, 0:O], borep[0:16, :])
        nc.sync.dma_start(y_d.ap(), osb[:])

    nc.compile()
    return nc


def _prep_inputs(x, Wi, bi, Wh, bh, Wo, bo):
    bf = ml_dtypes.bfloat16
    shared = {
        "wit": np.ascontiguousarray(Wi.T),
        "wh1t": np.ascontiguousarray(Wh[:, :H].T),
        "wh2b": np.ascontiguousarray(Wh[:, H:].T).astype(bf),
        "wob": np.ascontiguousarray(Wo.T).astype(bf),
        "bi": np.ascontiguousarray(bi.reshape(H // 128, 128).T),
        "bhrep": np.ascontiguousarray(np.broadcast_to(bh.reshape(1, H),
                                                      (128, H))),
        "borep": np.ascontiguousarray(np.broadcast_to(bo.reshape(1, O),
                                                      (128, O))),
        "eyeb": np.eye(128, dtype=np.float32).astype(bf),
    }
    in_maps = []
    for c in range(NCORES):
        xc = x[BL * c:BL * (c + 1)]            # [16, S, I]
        xt = np.ascontiguousarray(
            xc.transpose(2, 1, 0).reshape(I, xc.shape[1] * BL))
        m = dict(shared)
        m["xt"] = xt
        in_maps.append(m)
    return in_maps


def kernel(x, Wi, bi, Wh, bh, Wo, bo, _trace=False):
    global _CACHED
    x = np.asarray(x, dtype=np.float32)
    if _CACHED is None:
        _CACHED = _build()
    nc = _CACHED
    in_maps = _prep_inputs(np.asarray(x, np.float32), np.asarray(Wi, np.float32),
                           np.asarray(bi, np.float32), np.asarray(Wh, np.float32),
                           np.asarray(bh, np.float32), np.asarray(Wo, np.float32),
                           np.asarray(bo, np.float32))
    res = run_bass_kernel_spmd(nc, in_maps, list(range(NCORES)), trace=_trace)
    out = np.concatenate([res.results[c]["y"] for c in range(NCORES)], axis=0)
    if _trace:
        return out, res
    return out


# revision 35
# speedup vs baseline: 1.0248x; 1.0009x over previous
"""Trainium2 Bass kernel for a LeakyReLU RNN.

Model (B=128, S=512, I=256, H=1024, O=256):
    xproj = lrelu(x @ Wi.T + bi)                          # [B,S,H]
    h_t   = lrelu(concat(xproj_t, h_{t-1}) @ Wh.T + bh)   # recurrence over S
    out   = h_S @ Wo.T + bo                               # [B,O]

Strategy: data-parallel over batch (16 rows/core on 8 cores). Split
Wh = [Wh1 | Wh2]; U = xproj @ Wh1.T + bh is precomputed blockwise into
SBUF, the sequential part is h_t = lrelu(U_t + h_{t-1} @ Wh2.T).

The recurrence matmuls run in bf16 in 128x32 column-tiled PE mode:
three concurrent tiles (TRN2 forbids matmul dst partition 96), each
with its own moving-operand stream, so the per-step Wh2 stream is
~3072 cycles instead of 8192. Tile j covers output features
[384j, min(384j+384, 1024)) at PSUM partitions [32j, 32j+16). Phase-1
GEMM work for block b+2 is interleaved between the recurrence steps of
window b as PE filler, hiding phase 1 and keeping the PE p-state high.
bf16 is safe here: products accumulate in fp32 PSUM, U is injected
from an fp32-accumulated GEMM, and the recurrence is contractive
(||Wh2|| ~ 0.8), so the per-step ~4e-3 quantization error settles
around 1e-2, within the 2e-2 gate.
"""

from contextlib import ExitStack

import ml_dtypes
import numpy as np

import concourse.bacc as bacc
import concourse.tile as tile
from concourse import mybir
from concourse.bass_utils import run_bass_kernel_spmd

B, S, I, H, O = 128, 512, 256, 1024, 256
NCORES = 8
BL = B // NCORES          # batch rows per core = 16
TOK = BL * S              # tokens per core = 8192
ALPHA = 0.01
NSPLIT = (0, 384, 768, 1024)   # 3-way output-feature split for col tiles
# hT slot order: chunks whose transpose source lies in hn cols [0,256)
# first (they unblock after the first activation half), then the rest
KORDER = (0, 1, 3, 4, 6, 7, 2, 5)
import os
# ACT_SPLIT hangs the hardware: a partial-column activation read of a PSUM
# bank holding an active accumulation group never completes. Keep it off.
ACT_SPLIT = os.environ.get("ACT_SPLIT", "0") == "1"    # lrelu in two halves
COPY_SPLIT = os.environ.get("COPY_SPLIT", "0") == "1"  # psT -> ht in 2 copies
USE_KORDER = os.environ.get("USE_KORDER", "0") == "1"  # completion-order slots

F32 = mybir.dt.float32
F32R = mybir.dt.float32r
BF16 = mybir.dt.bfloat16
LRELU = mybir.ActivationFunctionType.Lrelu

_CACHED = None


def _build(S=S):
    TOK = BL * S
    NBLK = TOK // 512                 # 512-token (32-step) phase-1 blocks
    nc = bacc.Bacc("TRN2", target_bir_lowering=False, debug=False,
                   num_devices=NCORES)

    xt_d = nc.dram_tensor("xt", [I, TOK], F32, kind="ExternalInput")
    wit_d = nc.dram_tensor("wit", [I, H], F32, kind="ExternalInput")
    wh1t_d = nc.dram_tensor("wh1t", [H, H], F32, kind="ExternalInput")
    wh2b_d = nc.dram_tensor("wh2b", [H, H], BF16, kind="ExternalInput")
    wob_d = nc.dram_tensor("wob", [H, O], BF16, kind="ExternalInput")
    bi_d = nc.dram_tensor("bi", [128, H // 128], F32, kind="ExternalInput")
    bhrep_d = nc.dram_tensor("bhrep", [128, H], F32, kind="ExternalInput")
    borep_d = nc.dram_tensor("borep", [128, O], F32, kind="ExternalInput")
    eyeb_d = nc.dram_tensor("eyeb", [128, 128], BF16, kind="ExternalInput")
    y_d = nc.dram_tensor("y", [BL, O], F32, kind="ExternalOutput")

    with tile.TileContext(nc) as tc, ExitStack() as ctx:
        wpool = ctx.enter_context(tc.tile_pool(name="weights", bufs=1))
        xtpool = ctx.enter_context(tc.tile_pool(name="xt", bufs=2))
        apool = ctx.enter_context(tc.tile_pool(name="atiles", bufs=2))
        upool = ctx.enter_context(tc.tile_pool(name="usb", bufs=2))
        hnpool = ctx.enter_context(tc.tile_pool(name="hn", bufs=2))
        htpool = ctx.enter_context(tc.tile_pool(name="ht", bufs=2))
        opool = ctx.enter_context(tc.tile_pool(name="osb", bufs=1))
        psA = ctx.enter_context(tc.tile_pool(name="psA", bufs=2, space="PSUM"))
        psU = ctx.enter_context(tc.tile_pool(name="psU", bufs=1, space="PSUM"))
        psR = ctx.enter_context(tc.tile_pool(name="psR", bufs=2, space="PSUM"))
        psT = ctx.enter_context(tc.tile_pool(name="psT", bufs=2, space="PSUM"))

        # ---- resident weights ----
        def wload(src, shape, tag, dt=F32R, eng=None):
            t = wpool.tile(shape, dt, tag=tag, name=tag)
            (eng or nc.gpsimd).dma_start(t[:], src)
            return t

        wit = [wload(wit_d.ap()[128 * k:128 * (k + 1), :], [128, H], f"wit{k}")
               for k in range(2)]
        wh1t = [wload(wh1t_d.ap()[128 * k:128 * (k + 1), :], [128, H], f"wh1t{k}")
                for k in range(8)]
        wh2b = [wload(wh2b_d.ap()[128 * k:128 * (k + 1), :], [128, H],
                      f"wh2b{k}", dt=BF16, eng=nc.sync)
                for k in range(8)]
        wob = [wload(wob_d.ap()[128 * k:128 * (k + 1), :], [128, O],
                     f"wob{k}", dt=BF16, eng=nc.sync)
               for k in range(8)]
        eyeb = wload(eyeb_d.ap(), [128, 128], "eyeb", dt=BF16, eng=nc.sync)
        bhrep = wload(bhrep_d.ap(), [128, H], "bhrep", dt=F32, eng=nc.sync)
        borep = wload(borep_d.ap(), [128, O], "borep", dt=F32, eng=nc.sync)
        bi = wpool.tile([128, H // 128], F32, tag="bi", name="bi")
        nc.sync.dma_start(bi[:], bi_d.ap())

        # ---- phase 1 (emitted as filler chunks between recurrence steps)
        # Per 512-token block: A_T = lrelu(WiT.T @ Xt + bi)  (feature-major)
        # then U_g = A_g @ Wh1.T + bh for 4 groups of 128 tokens
        # (token-major [128 tok, 1024] bf16), kept in SBUF for the recurrence.
        usb = {}   # (blk % 2, g) -> SBUF tile [128, H]

        def phase1_chunks(blk, t_base):
            """Return a FIFO of (gate, closure); each closure emits
            ~200-450ns of engine work. A closure may only be emitted at
            absolute step >= gate: the usb evacuations overwrite (pool-wise)
            the buffer the current window's injects still read, so they are
            gated past the last consumer; the psU accumulations for the next
            group must stay behind the previous group's evacuation."""
            c0 = 512 * blk
            chunks = []
            xt = []
            a = []

            def dma_chunk():
                for k in range(2):
                    t = xtpool.tile([128, 512], F32R, tag=f"xt{k}",
                                    name=f"xt{k}_{blk}")
                    nc.gpsimd.dma_start(
                        t[:], xt_d.ap()[128 * k:128 * (k + 1), c0:c0 + 512])
                    xt.append(t)
            chunks.append((0, dma_chunk))

            def a_chunk(m):
                pa = psA.tile([128, 512], F32, tag="psA", name=f"psA_{blk}_{m}")
                nc.tensor.matmul(pa[:], wit[0][:, 128 * m:128 * (m + 1)],
                                 xt[0][:], start=True, stop=False)
                nc.tensor.matmul(pa[:], wit[1][:, 128 * m:128 * (m + 1)],
                                 xt[1][:], start=False, stop=True)
                am = apool.tile([128, 512], F32R, tag=f"a{m}", name=f"a{m}_{blk}")
                nc.scalar.activation(am[:], pa[:], LRELU,
                                     bias=bi[:, m:m + 1], scale=1.0, alpha=ALPHA)
                a.append(am)
            for m in range(8):
                chunks.append((0, lambda m=m: a_chunk(m)))

            # U for group g accumulates into psU [128 tok, 1024]
            # (each matmul dst must stay within one 2KB PSUM bank -> n-halves)
            pu = {}

            def u_chunk(g, k):
                if k == 0:
                    pu[g] = psU.tile([128, H], F32, tag="psU",
                                     name=f"psU_{blk}_{g}")
                for n in range(2):
                    nc.tensor.matmul(pu[g][:, 512 * n:512 * (n + 1)],
                                     a[k][:, 128 * g:128 * (g + 1)],
                                     wh1t[k][:, 512 * n:512 * (n + 1)],
                                     start=(k == 0), stop=(k == 7))

            def u_evac(g, q):
                # evacuate in 256-col slices so the DVE never blocks the
                # recurrence's critical hT copy for long; bf16 cast here
                if q == 0:
                    usb[(blk, g)] = upool.tile([128, H], BF16,
                                               tag=f"usb{g}",
                                               name=f"usb{g}_{blk}")
                sl = slice(256 * q, 256 * (q + 1))
                nc.vector.tensor_add(usb[(blk, g)][:, sl], pu[g][:, sl],
                                     bhrep[:, sl])

            for g in range(4):
                # psU has a single buffer: group g's matmuls may only be
                # emitted once group g-1's evacuation is fully emitted, which
                # the FIFO order plus the evac gates below guarantee.
                for k in range(8):
                    chunks.append((0, lambda g=g, k=k: u_chunk(g, k)))
                for q in range(4):
                    # the usb{g} buffer this evac overwrites is read by the
                    # current window's g-group injects (local steps 8g..8g+7)
                    chunks.append((t_base + 8 * g + 7,
                                   lambda g=g, q=q: u_evac(g, q)))
            return chunks

        # ---- recurrence state ----
        ht = htpool.tile([128, 128], BF16, tag="hT", name="hT_init")
        nc.gpsimd.memset(ht[:].bitcast(F32), 0.0)

        psq = {}   # t -> PSUM tile with U_t injected, one step ahead

        def emit_inject(t):
            """Inject U_t into a fresh PSUM tile (start of the accumulation
            group). Emitted right after the previous step's matmuls: same
            column-tiled PE mode, and it executes during the activation
            window instead of sitting in the transposes -> matmuls path."""
            blk, g, s = t // 32, (t % 32) // 8, t % 8
            ut = usb[(blk, g)]
            ps = psR.tile([128, 384], F32, tag="psR", name=f"psR_{t}")
            sel = eyeb[:, 16 * s:16 * (s + 1)]
            for j in range(3):
                w = NSPLIT[j + 1] - NSPLIT[j]
                nc.tensor.matmul(ps[32 * j:32 * j + 16, 0:w], sel,
                                 ut[:, NSPLIT[j]:NSPLIT[j + 1]],
                                 start=True, stop=False,
                                 tile_position=(0, 32 * j))
            psq[t] = ps

        def step(t):
            """One recurrence step: 3 col-tiles of bf16 matmuls + transposes.

            Col tile j covers features [NSPLIT[j], NSPLIT[j+1]) and writes
            PSUM partitions [32j, 32j+16)."""
            nonlocal ht
            ps = psq.pop(t)
            for o, k in enumerate(KORDER if USE_KORDER else range(8)):
                hk = ht[:, 16 * o:16 * (o + 1)]   # slot o holds chunk k
                for j in range(3):
                    w = NSPLIT[j + 1] - NSPLIT[j]
                    nc.tensor.matmul(ps[32 * j:32 * j + 16, 0:w], hk,
                                     wh2b[k][:, NSPLIT[j]:NSPLIT[j + 1]],
                                     start=False, stop=(o == 7),
                                     tile_position=(0, 32 * j))
            if t + 1 < S:
                emit_inject(t + 1)
            # lrelu split in two column halves so the first six transposes
            # (and the next step's first matmuls) start ~300ns earlier
            hn = hnpool.tile([128, 384], BF16, tag="hn", name=f"hn_{t}")
            if ACT_SPLIT:
                nc.scalar.activation(hn[:, 0:256], ps[:, 0:256], LRELU,
                                     bias=0.0, scale=1.0, alpha=ALPHA)
                nc.scalar.activation(hn[:, 256:384], ps[:, 256:384], LRELU,
                                     bias=0.0, scale=1.0, alpha=ALPHA)
            else:
                nc.scalar.activation(hn[:], ps[:], LRELU,
                                     bias=0.0, scale=1.0, alpha=ALPHA)
            # transpose back to feature-major stationary for the next step,
            # in completion order: chunks fed by the first act half first
            # (psT/ht slot o holds feature chunk KORDER[o])
            pt = psT.tile([128, 128], BF16, tag="psT", name=f"psT_{t}")
            for o, k in enumerate(KORDER if USE_KORDER else range(8)):
                j = (128 * k) // 384          # source col tile
                c = 128 * k - 384 * j         # col offset within tile j
                nc.tensor.transpose(
                    pt[:, 16 * o:16 * (o + 1)],
                    hn[32 * j:32 * j + 16, c:c + 128],
                    eyeb[32 * j:32 * j + 16, 32 * j:32 * j + 16],
                    tile_position=(32 * j, 0))
            ht_new = htpool.tile([128, 128], BF16, tag="hT", name=f"hT_{t}")
            if COPY_SPLIT:
                nc.vector.tensor_copy(ht_new[:, 0:96], pt[:, 0:96])
                nc.vector.tensor_copy(ht_new[:, 96:128], pt[:, 96:128])
            else:
                nc.vector.tensor_copy(ht_new[:], pt[:])
            ht = ht_new

        # ---- schedule: prologue blocks 0,1 then steps with filler ----
        for _, c in phase1_chunks(0, 0):
            c()
        if NBLK > 1:
            for _, c in phase1_chunks(1, 0):
                c()
        queue = []
        emit_inject(0)
        for t in range(S):
            if t % 32 == 0:
                nblk = t // 32 + 2
                if nblk < NBLK:
                    queue.extend(phase1_chunks(nblk, t))
            step(t)
            # emit filler chunks to keep the PE busy during the act/copy gap
            budget = 3
            while queue and budget > 0 and queue[0][0] <= t:
                queue.pop(0)[1]()
                budget -= 1

        # ---- phase 3: out = h_S @ Wo.T + bo ----
        po = psR.tile([128, 384], F32, tag="psR", name="psO")
        for o, k in enumerate(KORDER if USE_KORDER else range(8)):
            nc.tensor.matmul(po[0:16, 0:O], ht[:, 16 * o:16 * (o + 1)],
                             wob[k][:], start=(o == 0), stop=(o == 7))
        osb = opool.tile([16, O], F32, tag="osb", name="osb")
        nc.vector.tensor_add(osb[:], po[0:16, 0:O], borep[0:16, :])
        nc.sync.dma_start(y_d.ap(), osb[:])

    nc.compile()
    return nc


def _prep_inputs(x, Wi, bi, Wh, bh, Wo, bo):
    bf = ml_dtypes.bfloat16
    shared = {
        "wit": np.ascontiguousarray(Wi.T),
        "wh1t": np.ascontiguousarray(Wh[:, :H].T),
        "wh2b": np.ascontiguousarray(Wh[:, H:].T).astype(bf),
        "wob": np.ascontiguousarray(Wo.T).astype(bf),
        "bi": np.ascontiguousarray(bi.reshape(H // 128, 128).T),
        "bhrep": np.ascontiguousarray(np.broadcast_to(bh.reshape(1, H),
                                                      (128, H))),
        "borep": np.ascontiguousarray(np.broadcast_to(bo.reshape(1, O),
                                                      (128, O))),
        "eyeb": np.eye(128, dtype=np.float32).astype(bf),
    }
    in_maps = []
    for c in range(NCORES):
        xc = x[BL * c:BL * (c + 1)]            # [16, S, I]
        xt = np.ascontiguousarray(
            xc.transpose(2, 1, 0).reshape(I, xc.shape[1] * BL))
        m = dict(shared)
        m["xt"] = xt
        in_maps.append(m)
    return in_maps


def kernel(x, Wi, bi, Wh, bh, Wo, bo, _trace=False):
    global _CACHED
    x = np.asarray(x, dtype=np.float32)
    if _CACHED is None:
        _CACHED = _build()
    nc = _CACHED
    in_maps = _prep_inputs(np.asarray(x, np.float32), np.asarray(Wi, np.float32),
                           np.asarray(bi, np.float32), np.asarray(Wh, np.float32),
                           np.asarray(bh, np.float32), np.asarray(Wo, np.float32),
                           np.asarray(bo, np.float32))
    res = run_bass_kernel_spmd(nc, in_maps, list(range(NCORES)), trace=_trace)
    out = np.concatenate([res.results[c]["y"] for c in range(NCORES)], axis=0)
    if _trace:
        return out, res
    return out


# revision 36
# speedup vs baseline: 1.0343x; 1.0093x over previous
"""Trainium2 Bass kernel for a LeakyReLU RNN.

Model (B=128, S=512, I=256, H=1024, O=256):
    xproj = lrelu(x @ Wi.T + bi)                          # [B,S,H]
    h_t   = lrelu(concat(xproj_t, h_{t-1}) @ Wh.T + bh)   # recurrence over S
    out   = h_S @ Wo.T + bo                               # [B,O]

Strategy: data-parallel over batch (16 rows/core on 8 cores). Split
Wh = [Wh1 | Wh2]; U = xproj @ Wh1.T + bh is precomputed blockwise into
SBUF, the sequential part is h_t = lrelu(U_t + h_{t-1} @ Wh2.T).

The recurrence matmuls run in bf16 in 128x32 column-tiled PE mode:
three concurrent tiles (TRN2 forbids matmul dst partition 96), each
with its own moving-operand stream, so the per-step Wh2 stream is
~3072 cycles instead of 8192. Tile j covers output features
[384j, min(384j+384, 1024)) at PSUM partitions [32j, 32j+16). Phase-1
GEMM work for block b+2 is interleaved between the recurrence steps of
window b as PE filler, hiding phase 1 and keeping the PE p-state high.
bf16 is safe here: products accumulate in fp32 PSUM, U is injected
from an fp32-accumulated GEMM, and the recurrence is contractive
(||Wh2|| ~ 0.8), so the per-step ~4e-3 quantization error settles
around 1e-2, within the 2e-2 gate.
"""

from contextlib import ExitStack

import ml_dtypes
import numpy as np

import concourse.bacc as bacc
import concourse.tile as tile
from concourse import mybir
from concourse.bass_utils import run_bass_kernel_spmd

B, S, I, H, O = 128, 512, 256, 1024, 256
NCORES = 8
BL = B // NCORES          # batch rows per core = 16
TOK = BL * S              # tokens per core = 8192
ALPHA = 0.01
NSPLIT = (0, 384, 768, 1024)   # 3-way output-feature split for col tiles
# hT slot order: chunks whose transpose source lies in hn cols [0,256)
# first (they unblock after the first activation half), then the rest
KORDER = (0, 1, 3, 4, 6, 7, 2, 5)
import os
# ACT_SPLIT hangs the hardware: a partial-column activation read of a PSUM
# bank holding an active accumulation group never completes. Keep it off.
ACT_SPLIT = os.environ.get("ACT_SPLIT", "0") == "1"    # lrelu in two halves
COPY_SPLIT = os.environ.get("COPY_SPLIT", "0") == "1"  # psT -> ht in 2 copies
USE_KORDER = os.environ.get("USE_KORDER", "0") == "1"  # completion-order slots

F32 = mybir.dt.float32
F32R = mybir.dt.float32r
BF16 = mybir.dt.bfloat16
LRELU = mybir.ActivationFunctionType.Lrelu

_CACHED = None


def _build(S=S):
    TOK = BL * S
    NBLK = TOK // 512                 # 512-token (32-step) phase-1 blocks
    nc = bacc.Bacc("TRN2", target_bir_lowering=False, debug=False,
                   num_devices=NCORES)

    xt_d = nc.dram_tensor("xt", [I, TOK], F32, kind="ExternalInput")
    wit_d = nc.dram_tensor("wit", [I, H], F32, kind="ExternalInput")
    wh1t_d = nc.dram_tensor("wh1t", [H, H], F32, kind="ExternalInput")
    wh2b_d = nc.dram_tensor("wh2b", [H, H], BF16, kind="ExternalInput")
    wob_d = nc.dram_tensor("wob", [H, O], BF16, kind="ExternalInput")
    bi_d = nc.dram_tensor("bi", [128, H // 128], F32, kind="ExternalInput")
    bhrep_d = nc.dram_tensor("bhrep", [128, H], F32, kind="ExternalInput")
    borep_d = nc.dram_tensor("borep", [128, O], F32, kind="ExternalInput")
    eyeb_d = nc.dram_tensor("eyeb", [128, 128], BF16, kind="ExternalInput")
    y_d = nc.dram_tensor("y", [BL, O], F32, kind="ExternalOutput")

    with tile.TileContext(nc) as tc, ExitStack() as ctx:
        wpool = ctx.enter_context(tc.tile_pool(name="weights", bufs=1))
        xtpool = ctx.enter_context(tc.tile_pool(name="xt", bufs=2))
        apool = ctx.enter_context(tc.tile_pool(name="atiles", bufs=2))
        upool = ctx.enter_context(tc.tile_pool(name="usb", bufs=2))
        hnpool = ctx.enter_context(tc.tile_pool(name="hn", bufs=2))
        htpool = ctx.enter_context(tc.tile_pool(name="ht", bufs=2))
        opool = ctx.enter_context(tc.tile_pool(name="osb", bufs=1))
        psA = ctx.enter_context(tc.tile_pool(name="psA", bufs=2, space="PSUM"))
        psU = ctx.enter_context(tc.tile_pool(name="psU", bufs=1, space="PSUM"))
        psR = ctx.enter_context(tc.tile_pool(name="psR", bufs=2, space="PSUM"))
        psT = ctx.enter_context(tc.tile_pool(name="psT", bufs=2, space="PSUM"))

        # ---- resident weights ----
        def wload(src, shape, tag, dt=F32R, eng=None):
            t = wpool.tile(shape, dt, tag=tag, name=tag)
            (eng or nc.gpsimd).dma_start(t[:], src)
            return t

        wit = [wload(wit_d.ap()[128 * k:128 * (k + 1), :], [128, H], f"wit{k}")
               for k in range(2)]
        wh1t = [wload(wh1t_d.ap()[128 * k:128 * (k + 1), :], [128, H], f"wh1t{k}")
                for k in range(8)]
        wh2b = [wload(wh2b_d.ap()[128 * k:128 * (k + 1), :], [128, H],
                      f"wh2b{k}", dt=BF16, eng=nc.sync)
                for k in range(8)]
        wob = [wload(wob_d.ap()[128 * k:128 * (k + 1), :], [128, O],
                     f"wob{k}", dt=BF16, eng=nc.sync)
               for k in range(8)]
        eyeb = wload(eyeb_d.ap(), [128, 128], "eyeb", dt=BF16, eng=nc.sync)
        bhrep = wload(bhrep_d.ap(), [128, H], "bhrep", dt=F32, eng=nc.sync)
        borep = wload(borep_d.ap(), [128, O], "borep", dt=F32, eng=nc.sync)
        bi = wpool.tile([128, H // 128], F32, tag="bi", name="bi")
        nc.sync.dma_start(bi[:], bi_d.ap())

        # ---- phase 1 (emitted as filler chunks between recurrence steps)
        # Per 512-token block: A_T = lrelu(WiT.T @ Xt + bi)  (feature-major)
        # then U_g = A_g @ Wh1.T + bh for 4 groups of 128 tokens
        # (token-major [128 tok, 1024] bf16), kept in SBUF for the recurrence.
        usb = {}   # (blk % 2, g) -> SBUF tile [128, H]

        def phase1_chunks(blk, t_base):
            """Return a FIFO of (gate, closure); each closure emits
            ~200-450ns of engine work. A closure may only be emitted at
            absolute step >= gate: the usb evacuations overwrite (pool-wise)
            the buffer the current window's injects still read, so they are
            gated past the last consumer; the psU accumulations for the next
            group must stay behind the previous group's evacuation."""
            c0 = 512 * blk
            chunks = []
            xt = []
            a = []

            def dma_chunk():
                for k in range(2):
                    t = xtpool.tile([128, 512], F32R, tag=f"xt{k}",
                                    name=f"xt{k}_{blk}")
                    nc.gpsimd.dma_start(
                        t[:], xt_d.ap()[128 * k:128 * (k + 1), c0:c0 + 512])
                    xt.append(t)
            chunks.append((0, dma_chunk))

            def a_chunk(m):
                pa = psA.tile([128, 512], F32, tag="psA", name=f"psA_{blk}_{m}")
                nc.tensor.matmul(pa[:], wit[0][:, 128 * m:128 * (m + 1)],
                                 xt[0][:], start=True, stop=False)
                nc.tensor.matmul(pa[:], wit[1][:, 128 * m:128 * (m + 1)],
                                 xt[1][:], start=False, stop=True)
                am = apool.tile([128, 512], F32R, tag=f"a{m}", name=f"a{m}_{blk}")
                nc.scalar.activation(am[:], pa[:], LRELU,
                                     bias=bi[:, m:m + 1], scale=1.0, alpha=ALPHA)
                a.append(am)
            for m in range(8):
                chunks.append((0, lambda m=m: a_chunk(m)))

            # U for group g accumulates into psU [128 tok, 1024]
            # (each matmul dst must stay within one 2KB PSUM bank -> n-halves)
            pu = {}

            def u_chunk(g, k):
                if k == 0:
                    pu[g] = psU.tile([128, H], F32, tag="psU",
                                     name=f"psU_{blk}_{g}")
                for n in range(2):
                    nc.tensor.matmul(pu[g][:, 512 * n:512 * (n + 1)],
                                     a[k][:, 128 * g:128 * (g + 1)],
                                     wh1t[k][:, 512 * n:512 * (n + 1)],
                                     start=(k == 0), stop=(k == 7))

            def u_evac(g, q):
                # evacuate in 256-col slices so the DVE never blocks the
                # recurrence's critical hT copy for long; bf16 cast here
                if q == 0:
                    usb[(blk, g)] = upool.tile([128, H], BF16,
                                               tag=f"usb{g}",
                                               name=f"usb{g}_{blk}")
                sl = slice(256 * q, 256 * (q + 1))
                nc.vector.tensor_add(usb[(blk, g)][:, sl], pu[g][:, sl],
                                     bhrep[:, sl])

            for g in range(4):
                # psU has a single buffer: group g's matmuls may only be
                # emitted once group g-1's evacuation is fully emitted, which
                # the FIFO order plus the evac gates below guarantee.
                for k in range(8):
                    chunks.append((0, lambda g=g, k=k: u_chunk(g, k)))
                for q in range(4):
                    # the usb{g} buffer this evac overwrites is read by the
                    # current window's g-group injects (local steps 8g..8g+7)
                    chunks.append((t_base + 8 * g + 7,
                                   lambda g=g, q=q: u_evac(g, q)))
            return chunks

        # ---- recurrence state ----
        ht = htpool.tile([128, 128], BF16, tag="hT", name="hT_init")
        nc.gpsimd.memset(ht[:].bitcast(F32), 0.0)

        psq = {}   # t -> PSUM tile with U_t injected, one step ahead

        def emit_inject(t):
            """Inject U_t into a fresh PSUM tile (start of the accumulation
            group). Emitted right after the previous step's matmuls: same
            column-tiled PE mode, and it executes during the activation
            window instead of sitting in the transposes -> matmuls path."""
            blk, g, s = t // 32, (t % 32) // 8, t % 8
            ut = usb[(blk, g)]
            ps = psR.tile([128, 384], F32, tag="psR", name=f"psR_{t}")
            sel = eyeb[:, 16 * s:16 * (s + 1)]
            for j in range(3):
                w = NSPLIT[j + 1] - NSPLIT[j]
                nc.tensor.matmul(ps[32 * j:32 * j + 16, 0:w], sel,
                                 ut[:, NSPLIT[j]:NSPLIT[j + 1]],
                                 start=True, stop=False,
                                 tile_position=(0, 32 * j))
            psq[t] = ps

        def step(t):
            """One recurrence step: 3 col-tiles of bf16 matmuls + transposes.

            Col tile j covers features [NSPLIT[j], NSPLIT[j+1]) and writes
            PSUM partitions [32j, 32j+16)."""
            nonlocal ht
            ps = psq.pop(t)
            for o, k in enumerate(KORDER if USE_KORDER else range(8)):
                hk = ht[:, 16 * o:16 * (o + 1)]   # slot o holds chunk k
                for j in range(3):
                    w = NSPLIT[j + 1] - NSPLIT[j]
                    nc.tensor.matmul(ps[32 * j:32 * j + 16, 0:w], hk,
                                     wh2b[k][:, NSPLIT[j]:NSPLIT[j + 1]],
                                     start=False, stop=(o == 7),
                                     tile_position=(0, 32 * j))
            if t + 1 < S:
                emit_inject(t + 1)
            # lrelu split in two column halves so the first six transposes
            # (and the next step's first matmuls) start ~300ns earlier
            hn = hnpool.tile([128, 384], BF16, tag="hn", name=f"hn_{t}")
            if ACT_SPLIT:
                nc.scalar.activation(hn[:, 0:256], ps[:, 0:256], LRELU,
                                     bias=0.0, scale=1.0, alpha=ALPHA)
                nc.scalar.activation(hn[:, 256:384], ps[:, 256:384], LRELU,
                                     bias=0.0, scale=1.0, alpha=ALPHA)
            else:
                nc.scalar.activation(hn[:], ps[:], LRELU,
                                     bias=0.0, scale=1.0, alpha=ALPHA)
            # transpose back to feature-major stationary for the next step,
            # in completion order: chunks fed by the first act half first
            # (psT/ht slot o holds feature chunk KORDER[o])
            pt = psT.tile([128, 128], BF16, tag="psT", name=f"psT_{t}")
            for o, k in enumerate(KORDER if USE_KORDER else range(8)):
                j = (128 * k) // 384          # source col tile
                c = 128 * k - 384 * j         # col offset within tile j
                nc.tensor.transpose(
                    pt[:, 16 * o:16 * (o + 1)],
                    hn[32 * j:32 * j + 16, c:c + 128],
                    eyeb[32 * j:32 * j + 16, 32 * j:32 * j + 16],
                    tile_position=(32 * j, 0))
            ht_new = htpool.tile([128, 128], BF16, tag="hT", name=f"hT_{t}")
            if COPY_SPLIT:
                nc.vector.tensor_copy(ht_new[:, 0:96], pt[:, 0:96])
                nc.vector.tensor_copy(ht_new[:, 96:128], pt[:, 96:128])
            else:
                nc.vector.tensor_copy(ht_new[:], pt[:])
            ht = ht_new

        # ---- schedule: prologue block 0, then steps with filler ----
        # block w+1 is filled during window w: its usb buffers have the
        # opposite parity to window w's reads, so no same-window conflict,
        # and its evacuations land a full window before their consumers
        for _, c in phase1_chunks(0, 0):
            c()
        queue = []
        emit_inject(0)
        for t in range(S):
            if t % 32 == 0:
                nblk = t // 32 + 1
                if 1 <= nblk < NBLK:
                    queue.extend(phase1_chunks(nblk, t))
            step(t)
            # emit filler chunks to keep the PE busy during the act/copy gap
            budget = 3
            while queue and budget > 0 and queue[0][0] <= t:
                queue.pop(0)[1]()
                budget -= 1

        # ---- phase 3: out = h_S @ Wo.T + bo ----
        po = psR.tile([128, 384], F32, tag="psR", name="psO")
        for o, k in enumerate(KORDER if USE_KORDER else range(8)):
            nc.tensor.matmul(po[0:16, 0:O], ht[:, 16 * o:16 * (o + 1)],
                             wob[k][:], start=(o == 0), stop=(o == 7))
        osb = opool.tile([16, O], F32, tag="osb", name="osb")
        nc.vector.tensor_add(osb[:], po[0:16, 0:O], borep[0:16, :])
        nc.sync.dma_start(y_d.ap(), osb[:])

    nc.compile()
    return nc


def _prep_inputs(x, Wi, bi, Wh, bh, Wo, bo):
    bf = ml_dtypes.bfloat16
    shared = {
        "wit": np.ascontiguousarray(Wi.T),
        "wh1t": np.ascontiguousarray(Wh[:, :H].T),
        "wh2b": np.ascontiguousarray(Wh[:, H:].T).astype(bf),
        "wob": np.ascontiguousarray(Wo.T).astype(bf),
        "bi": np.ascontiguousarray(bi.reshape(H // 128, 128).T),
        "bhrep": np.ascontiguousarray(np.broadcast_to(bh.reshape(1, H),
                                                      (128, H))),
        "borep": np.ascontiguousarray(np.broadcast_to(bo.reshape(1, O),
                                                      (128, O))),
        "eyeb": np.eye(128, dtype=np.float32).astype(bf),
    }
    in_maps = []
    for c in range(NCORES):
        xc = x[BL * c:BL * (c + 1)]            # [16, S, I]
        xt = np.ascontiguousarray(
            xc.transpose(2, 1, 0).reshape(I, xc.shape[1] * BL))
        m = dict(shared)
        m["xt"] = xt
        in_maps.append(m)
    return in_maps


def kernel(x, Wi, bi, Wh, bh, Wo, bo, _trace=False):
    global _CACHED
    x = np.asarray(x, dtype=np.float32)
    if _CACHED is None:
        _CACHED = _build()
    nc = _CACHED
    in_maps = _prep_inputs(np.asarray(x, np.float32), np.asarray(Wi, np.float32),
                           np.asarray(bi, np.float32), np.asarray(Wh, np.float32),
                           np.asarray(bh, np.float32), np.asarray(Wo, np.float32),
                           np.asarray(bo, np.float32))
    res = run_bass_kernel_spmd(nc, in_maps, list(range(NCORES)), trace=_trace)
    out = np.concatenate([res.results[c]["y"] for c in range(NCORES)], axis=0)
    if _trace:
        return out, res
    return out
